# revision 1
# baseline (speedup 1.0000x reference)
"""Trainium2 Bass kernel for nn_ContrastiveCenterLoss_M.

Math reduction
--------------
reference computes, per sample b and class c, a Mahalanobis distance between
the pooled-normalized hidden vector x_b (8-dim) and pooled-normalized class
center y_c (8-dim), where the 8x8 covariance is over the 200 points
{x_b (repeated 100x), y_0..y_99}:

    cov_b = A + beta d_b d_b^T,  A = S_y/199,  d_b = x_b - ybar,  beta = 50/199

A depends only on feature_center and is well-conditioned (cond ~1.9), so
pinv == inv and Sherman-Morrison collapses the per-sample pinv to a rank-1
correction of the shared M = inv(A).  Working in u = x - ybar coordinates
(all class-only terms folded into host constants):

    ur_c  = u.(M y_c)                     [one 9x128^T @ 9x209 matmul]
    e0    = u.(M ybar)                    [extra matmul column]
    uw    = u.M.u = x.w - e0,  w = M u    [w = 8 extra matmul columns]
    gamma = 1/(1/beta + uw),   sg = sqrt(gamma)
    m[b,c] = (uw + 2 e0) + (k2_c - 2 ur_c) - (sg*ur_c - sg*(uw+e0))^2
    k2_c  = (y_c-ybar).M.(y_c-ybar)       [host]
    dis = sqrt(m);  loss_b = (C*dis[b,y_b] - sum_c dis[b,c])/(C-1)

Host precomputes the tiny center-only constants in float64; the device does
all per-sample work.  Data-parallel over batch: 8 cores x 128 samples.
ACT-table sqrt measured at ~1e-6 rel on HW, so no Newton refinement.
NOTE: InstTensorTensorReduce and [p,1]-shaped DRAM outputs crash the exec
unit on this runtime -- avoided (tt+reduce pairs; [128,100] output).
"""

import sys

if "/opt/trn_rl_repo" not in sys.path:
    sys.path.insert(0, "/opt/trn_rl_repo")

import numpy as np

B = 1024
D = 512
C = 100
POOL = 8
G = D // POOL          # 64
NCORES = 8
BS = B // NCORES       # 128 samples per core
BETA = (C / 2) / (2 * C - 1)   # 50/199
NCONST = 1 + 209       # [ybar9 | rhsU(9x209)] packed columns

_cache = {}


def _build():
    import concourse.mybir as mybir
    import concourse.tile as tile
    from concourse import bacc
    from concourse.masks import make_identity

    f32 = mybir.dt.float32
    ALU = mybir.AluOpType
    ACT = mybir.ActivationFunctionType
    AX = mybir.AxisListType
    HALF = D // 2

    nc = bacc.Bacc(
        "TRN2",
        target_bir_lowering=False,
        debug=False,
        enable_asserts=False,
        num_devices=NCORES,
    )

    hidden_d = nc.dram_tensor("hidden_in", [BS, D + 1], f32, kind="ExternalInput")
    const_d = nc.dram_tensor("const_in", [POOL + 1, NCONST], f32, kind="ExternalInput")
    loss_d = nc.dram_tensor("loss_out", [BS, C], f32, kind="ExternalOutput")

    with tile.TileContext(nc) as tc:
        with (
            tc.tile_pool(name="sb", bufs=1) as sb,
            tc.tile_pool(name="ps", bufs=1, space="PSUM") as ps,
        ):
            # ACT-table ordering hint: make the first ACT op a Sqrt so walrus
            # loads the sqrt set (which also contains square) exactly once,
            # early, overlapped with the DMA.
            warm = sb.tile([1, 1], f32)
            nc.vector.memset(warm[:, :], 1.0)
            nc.scalar.sqrt(out=warm[:, :], in_=warm[:, :])

            # hidden + labels (y packed as f32 col 512), asymmetric split: the
            # second (later-arriving) DMA is kept small so pooling finishes sooner
            SP1 = 6 * G
            h1 = sb.tile([BS, SP1], f32)
            h2 = sb.tile([BS, D - SP1 + 1], f32)
            nc.sync.dma_start(h1[:, :], hidden_d[:, 0:SP1])
            nc.sync.dma_start(h2[:, :], hidden_d[:, SP1:D + 1])
            ylab = h2[:, D - SP1:D - SP1 + 1]
            cst = sb.tile([POOL + 1, NCONST], f32)
            nc.sync.dma_start(cst[:, :], const_d[:, :])
            nybar9 = cst[:, 0:1]   # holds -ybar
            rhsU = cst[:, 1:1 + 209]

            # constants with no deps: identity (PE transpose) + iota (one-hot)
            ident = sb.tile([BS, BS], f32)
            make_identity(nc, ident[:, :])
            io_f = sb.tile([BS, C], f32)
            nc.gpsimd.iota(out=io_f[:, :], pattern=[[1, C]], base=0,
                           channel_multiplier=0, allow_small_or_imprecise_dtypes=True)

            # ---- pool hidden into 8 groups of 64, L2-normalize -> x ----
            s8 = sb.tile([BS, POOL], f32)
            nc.vector.tensor_reduce(
                out=s8[:, 0:6],
                in_=h1[:, :].rearrange("p (g e) -> p g e", e=G),
                axis=AX.X, op=ALU.add,
            )
            nc.vector.tensor_reduce(
                out=s8[:, 6:POOL],
                in_=h2[:, 0:D - SP1].rearrange("p (g e) -> p g e", e=G),
                axis=AX.X, op=ALU.add,
            )
            sq = sb.tile([BS, POOL], f32)
            ss = sb.tile([BS, 1], f32)
            nc.scalar.activation(
                out=sq[:, :], in_=s8[:, :], func=ACT.Square, scale=1.0 / G,
                accum_out=ss[:, :],
            )
            nv = sb.tile([BS, 1], f32)
            nc.scalar.sqrt(out=nv[:, :], in_=ss[:, :])
            ne = sb.tile([BS, 1], f32)
            nc.vector.tensor_scalar(out=ne[:, :], in0=nv[:, :], scalar1=1e-6,
                                    scalar2=None, op0=ALU.add)
            rn = sb.tile([BS, 1], f32)
            nc.vector.reciprocal(out=rn[:, :], in_=ne[:, :])
            xn9 = sb.tile([BS, POOL + 1], f32)   # [x | 1]
            nc.vector.tensor_scalar(
                out=xn9[:, 0:POOL], in0=s8[:, :], scalar1=1.0 / G, scalar2=rn[:, 0:1],
                op0=ALU.mult, op1=ALU.mult,
            )
            nc.vector.memset(xn9[:, POOL:POOL + 1], 1.0)

            # one-hot mask of the true class: off the critical path
            oh = sb.tile([BS, C], f32)
            nc.vector.tensor_scalar(out=oh[:, :], in0=io_f[:, :], scalar1=ylab[:, 0:1],
                                    scalar2=None, op0=ALU.is_equal)

            # ---- u^T (+ones row) = transpose(x|1) - (ybar|0) ----
            xnt_ps = ps.tile([POOL + 1, BS], f32)
            nc.tensor.transpose(xnt_ps[:, :], xn9[:, :], ident[:, :])
            ut9 = sb.tile([POOL + 1, BS], f32)
            nc.scalar.activation(out=ut9[:, :], in_=xnt_ps[:, :], func=ACT.Identity,
                                 bias=nybar9)

            # ---- the contraction, split so the small w/e0 block lands first
            # and the per-sample scalar chain overlaps the big 200-col matmul.
            # wps col 0 = e0 = u.(M ybar) ; cols 1:9 = w = M u
            # dis_ps cols 0:100 = u.r_c ; 100:200 = k2_c - 2 u.r_c
            wps = ps.tile([BS, 9], f32)
            nc.tensor.matmul(wps[:, :], ut9[:, :], rhsU[:, 200:209])
            dis_ps = ps.tile([BS, 200], f32)
            nc.tensor.matmul(dis_ps[:, :], ut9[:, :], rhsU[:, 0:200])

            # ---- per-sample scalars (note uw + e0 = u.M.x = x.w = xws) ----
            xw = sb.tile([BS, POOL], f32)
            nc.vector.tensor_tensor(out=xw[:, :], in0=xn9[:, 0:POOL],
                                    in1=wps[:, 1:9], op=ALU.mult)
            xws = sb.tile([BS, 1], f32)
            nc.vector.tensor_reduce(out=xws[:, :], in_=xw[:, :], axis=AX.X, op=ALU.add)
            e0 = sb.tile([BS, 1], f32)
            nc.vector.tensor_copy(out=e0[:, :], in_=wps[:, 0:1])
            den = sb.tile([BS, 1], f32)    # 1/beta + uw = (xws - e0) + 1/beta
            nc.vector.tensor_scalar(out=den[:, :], in0=xws[:, :], scalar1=e0[:, 0:1],
                                    scalar2=1.0 / BETA, op0=ALU.subtract, op1=ALU.add)
            gam = sb.tile([BS, 1], f32)
            nc.vector.reciprocal(out=gam[:, :], in_=den[:, :])
            s2 = sb.tile([BS, 1], f32)     # uw + 2 e0 = xws + e0
            nc.vector.tensor_scalar(out=s2[:, :], in0=xws[:, :], scalar1=e0[:, 0:1],
                                    scalar2=None, op0=ALU.add)

            # ---- m = (k2 - 2ur) - (gam*(ur - xws)^2 - s2) ; dis = sqrt(m) ----
            nxws = sb.tile([BS, 1], f32)
            nc.vector.tensor_scalar(out=nxws[:, :], in0=xws[:, :], scalar1=-1.0,
                                    scalar2=None, op0=ALU.mult)
            qsq = sb.tile([BS, C], f32)     # (ur - xws)^2 on ACT, parallel with DVE
            nc.scalar.activation(out=qsq[:, :], in_=dis_ps[:, 0:C], func=ACT.Square,
                                 bias=nxws[:, 0:1])
            gq2 = sb.tile([BS, C], f32)     # gam*qsq - s2
            nc.vector.tensor_scalar(out=gq2[:, :], in0=qsq[:, :], scalar1=gam[:, 0:1],
                                    scalar2=s2[:, 0:1], op0=ALU.mult, op1=ALU.subtract)
            m = sb.tile([BS, C], f32)
            nc.vector.tensor_tensor(out=m[:, :], in0=dis_ps[:, C:2 * C],
                                    in1=gq2[:, :], op=ALU.subtract)

            # rowsum via ACT accum; true-class element via mask+reduce (parallel)
            dis = sb.tile([BS, C], f32)
            rowsum = sb.tile([BS, 1], f32)
            nc.scalar.activation(out=dis[:, :], in_=m[:, :], func=ACT.Sqrt,
                                 accum_out=rowsum[:, :])
            mh = sb.tile([BS, C], f32)
            nc.vector.tensor_tensor(out=mh[:, :], in0=m[:, :], in1=oh[:, :], op=ALU.mult)
            mt = sb.tile([BS, 1], f32)
            nc.vector.tensor_reduce(out=mt[:, :], in_=mh[:, :], axis=AX.X, op=ALU.add)
            intra = sb.tile([BS, 1], f32)
            nc.scalar.sqrt(out=intra[:, :], in_=mt[:, :])
            rs1 = sb.tile([BS, 1], f32)
            nc.vector.tensor_scalar(out=rs1[:, :], in0=rowsum[:, :], scalar1=-1.0 / (C - 1),
                                    scalar2=None, op0=ALU.mult)
            loss = sb.tile([BS, 1], f32)
            nc.vector.tensor_scalar(out=loss[:, :], in0=intra[:, :],
                                    scalar1=float(C) / (C - 1), scalar2=rs1[:, 0:1],
                                    op0=ALU.mult, op1=ALU.add)
            nc.sync.dma_start(loss_d[:, 0:1], loss[:, :])

    nc.finalize()
    return nc


def _get_nc():
    if "nc" not in _cache:
        _cache["nc"] = _build()
    return _cache["nc"]


def _host_precompute(feature_center):
    fc = np.asarray(feature_center, dtype=np.float64)
    g = fc.reshape(C, POOL, G).mean(axis=2)                  # [100, 8]
    yn = g / (np.linalg.norm(g, axis=1, keepdims=True) + 1e-6)
    ybar = yn.mean(axis=0)
    z = yn - ybar
    A = (z.T @ z) / (2 * C - 1)
    M = np.linalg.inv(A)
    M = 0.5 * (M + M.T)
    r = yn @ M                                               # [100, 8]  M y_c
    c0 = M @ ybar
    k2 = np.einsum('cd,ce,de->c', z, z, M)                   # z_c M z_c

    cp = np.zeros((POOL + 1, NCONST), dtype=np.float64)
    cp[0:POOL, 0] = -ybar
    cp[0:POOL, 1:1 + C] = r.T
    cp[0:POOL, 1 + C:1 + 2 * C] = -2.0 * r.T
    cp[POOL, 1 + C:1 + 2 * C] = k2
    cp[0:POOL, 1 + 2 * C] = c0
    cp[0:POOL, 1 + 2 * C + 1:1 + 2 * C + 1 + POOL] = M
    return cp.astype(np.float32)


def kernel(hidden, feature_center, y):
    from concourse import bass_utils

    ha = np.empty((B, D + 1), dtype=np.float32)
    ha[:, 0:D] = np.asarray(hidden, dtype=np.float32)
    ha[:, D] = np.asarray(y).astype(np.float32)
    cp = _host_precompute(feature_center)

    nc = _get_nc()
    in_maps = []
    for c in range(NCORES):
        in_maps.append({
            "hidden_in": ha[c * BS:(c + 1) * BS],
            "const_in": cp,
        })
    res = bass_utils.run_bass_kernel_spmd(nc, in_maps, core_ids=list(range(NCORES)))
    loss = np.concatenate([r["loss_out"][:, 0] for r in res.results])
    return np.float32(loss.mean())



# revision 3
# speedup vs baseline: 1.1616x; 1.1616x over previous
"""Trainium2 Bass kernel for nn_ContrastiveCenterLoss_M.

Math reduction
--------------
reference computes, per sample b and class c, a Mahalanobis distance between
the pooled-normalized hidden vector x_b (8-dim) and pooled-normalized class
center y_c (8-dim), where the 8x8 covariance is over the 200 points
{x_b (repeated 100x), y_0..y_99}:

    cov_b = A + beta d_b d_b^T,  A = S_y/199,  d_b = x_b - ybar,  beta = 50/199

A depends only on feature_center and is well-conditioned, so pinv == inv and
Sherman-Morrison collapses the per-sample pinv to a rank-1 correction of the
shared M = inv(A).  With u = x - ybar and all class-side constants folded
into the matmul's ones-row (host precompute, float64):

    wps  = [x|1] @ [M; -c0 | c0; -k0]          ->  w = M u (8 cols), e0 (1 col)
    dis_ = [x|1] @ [r; -kr | -2r; k2+2kr]      ->  ur (100), k2-2ur (100)
    nxws = -sum(x*w); ngam = 1/(nxws+e0-1/beta); s2 = -(nxws-e0)
    m    = (k2-2ur) + ngam*(ur+nxws)^2 + s2;   dis = sqrt(m)
    loss_b = sum_c dis * mask2_c,  mask2 = onehot*C/(C-1) - 1/(C-1)

Everything device-side runs bf16 where it feeds the PE (validated 8e-4 final
rel err in numpy emulation): bf16 hidden DMA (halves HBM bytes), bf16 pooled
sums, DVE 32x32 stream-transposes build the [9,128] lhsT in SBUF directly
(no PE transpose + PSUM round trip), bf16 matmuls (1 cyc/row vs 4 for f32).
1/|s| via DVE pow(-0.5) (eps dropped: |s|~22 >> 64e-6).  Scalar tail fused
into scalar_tensor_tensor ops with accum_out; the final loss is a dis.mask2
dot with accumulate.  Data-parallel over batch: 8 cores x 128 samples.
NOTE: [p,1]-shaped DRAM outputs crash the exec unit -- loss_out is [128,100]
with only col 0 written (same workaround as v1).
"""

import sys

if "/opt/trn_rl_repo" not in sys.path:
    sys.path.insert(0, "/opt/trn_rl_repo")

import numpy as np

B = 1024
D = 512
C = 100
POOL = 8
G = D // POOL          # 64
NCORES = 8
BS = B // NCORES       # 128 samples per core
BETA = (C / 2) / (2 * C - 1)   # 50/199

_cache = {}


def _build():
    import concourse.mybir as mybir
    import concourse.tile as tile
    from concourse import bacc

    f32 = mybir.dt.float32
    bf16 = mybir.dt.bfloat16
    ALU = mybir.AluOpType
    ACT = mybir.ActivationFunctionType
    AX = mybir.AxisListType

    nc = bacc.Bacc(
        "TRN2",
        target_bir_lowering=False,
        debug=False,
        enable_asserts=False,
        num_devices=NCORES,
    )

    hidden_d = nc.dram_tensor("hidden_in", [BS, D + 1], bf16, kind="ExternalInput")
    const_d = nc.dram_tensor("const_in", [POOL + 1, 209], bf16, kind="ExternalInput")
    loss_d = nc.dram_tensor("loss_out", [BS, C], f32, kind="ExternalOutput")

    with tile.TileContext(nc) as tc:
        with (
            tc.tile_pool(name="sb", bufs=1) as sb,
            tc.tile_pool(name="ps", bufs=1, space="PSUM") as ps,
        ):
            # ACT-table hint: first ACT op is a Sqrt so the sqrt set (which
            # also contains square) loads once, overlapped with the DMA.
            warm = sb.tile([1, 1], f32)
            nc.vector.memset(warm[:, :], 1.0)
            nc.scalar.sqrt(out=warm[:, :], in_=warm[:, :])

            # single bf16 DMA: 512 hidden cols + y as col 512
            h = sb.tile([BS, D + 1], bf16)
            nc.sync.dma_start(h[:, :], hidden_d[:, :])
            cst = sb.tile([POOL + 1, 209], bf16)
            nc.sync.dma_start(cst[:, :], const_d[:, :])

            # xn: [x | 1] padded to 32 cols for the 32x32 stream transpose.
            # memset 1.0 covers the ones col (8) and harmless pad cols.
            xn = sb.tile([BS, 32], bf16)
            nc.vector.memset(xn[:, :], 1.0)

            # one-hot -> mask2 on the (otherwise idle) Pool engine
            io_f = sb.tile([BS, C], f32)
            nc.gpsimd.iota(out=io_f[:, :], pattern=[[1, C]], base=0,
                           channel_multiplier=0, allow_small_or_imprecise_dtypes=True)
            ylab = sb.tile([BS, 1], f32)
            nc.gpsimd.tensor_copy(out=ylab[:, :], in_=h[:, D:D + 1])
            oh = sb.tile([BS, C], f32)
            nc.gpsimd.tensor_scalar(out=oh[:, :], in0=io_f[:, :], scalar1=ylab[:, 0:1],
                                    scalar2=None, op0=ALU.is_equal)
            mask2 = sb.tile([BS, C], f32)
            nc.gpsimd.tensor_scalar(out=mask2[:, :], in0=oh[:, :],
                                    scalar1=float(C) / (C - 1),
                                    scalar2=-1.0 / (C - 1), op0=ALU.mult, op1=ALU.add)

            # ---- pool hidden into 8 group sums (bf16), L2-normalize ----
            s8 = sb.tile([BS, POOL], bf16)
            with nc.allow_low_precision(reason="bf16 pooled sums; 8e-4 final rel err"):
                nc.vector.tensor_reduce(
                    out=s8[:, :],
                    in_=h[:, 0:D].rearrange("p (g e) -> p g e", e=G),
                    axis=AX.X, op=ALU.add,
                )
            sq = sb.tile([BS, POOL], f32)
            ss = sb.tile([BS, 1], f32)
            nc.vector.scalar_tensor_tensor(
                out=sq[:, :], in0=s8[:, :], scalar=1.0, in1=s8[:, :],
                op0=ALU.mult, op1=ALU.mult, accum_out=ss[:, :],
            )
            nv = sb.tile([BS, 1], f32)
            nc.scalar.sqrt(out=nv[:, :], in_=ss[:, :])
            rn = sb.tile([BS, 1], f32)
            nc.vector.reciprocal(out=rn[:, :], in_=nv[:, :])
            nc.vector.tensor_scalar(out=xn[:, 0:POOL], in0=s8[:, :], scalar1=rn[:, 0:1],
                                    scalar2=None, op0=ALU.mult)

            # ---- lhsT [9,128] via 4 DVE 32x32 block transposes (SBUF->SBUF) ----
            xnt = sb.tile([32, BS], bf16)
            for q in range(4):
                nc.vector.transpose(out=xnt[0:32, 32 * q:32 * q + 32],
                                    in_=xn[32 * q:32 * q + 32, 0:32])
            # f32 copy of x for the nxws dot (avoids mixed bf16/psum-f32 stt)
            xnf = sb.tile([BS, POOL], f32)
            nc.vector.tensor_copy(out=xnf[:, :], in_=xn[:, 0:POOL])

            # ---- matmuls: w|e0 block first so the scalar chain overlaps ----
            wps = ps.tile([BS, POOL + 1], f32)
            nc.tensor.matmul(wps[:, :], xnt[0:POOL + 1, :], cst[:, 0:POOL + 1])
            dis_ps = ps.tile([BS, 2 * C], f32)
            nc.tensor.matmul(dis_ps[:, :], xnt[0:POOL + 1, :], cst[:, POOL + 1:209])

            # ---- per-sample scalars (negated forms save negation ops) ----
            sqj = sb.tile([BS, POOL], f32)
            nxws = sb.tile([BS, 1], f32)      # -x.w = -(uw+e0)
            nc.vector.scalar_tensor_tensor(
                out=sqj[:, :], in0=xnf[:, :], scalar=-1.0, in1=wps[:, 0:POOL],
                op0=ALU.mult, op1=ALU.mult, accum_out=nxws[:, :],
            )
            dent = sb.tile([BS, 1], f32)      # (nxws + e0) - 1/beta = -den
            nc.vector.tensor_scalar(out=dent[:, :], in0=nxws[:, :],
                                    scalar1=wps[:, POOL:POOL + 1],
                                    scalar2=1.0 / BETA, op0=ALU.add, op1=ALU.subtract)
            ngam = sb.tile([BS, 1], f32)      # -gamma
            nc.vector.reciprocal(out=ngam[:, :], in_=dent[:, :])
            s2 = sb.tile([BS, 1], f32)        # xws + e0 = uw + 2 e0
            nc.vector.tensor_scalar(out=s2[:, :], in0=nxws[:, :],
                                    scalar1=wps[:, POOL:POOL + 1],
                                    scalar2=-1.0, op0=ALU.subtract, op1=ALU.mult)

            # ---- m = (k2-2ur) + ngam*(ur-xws)^2 + s2 ; dis = sqrt(m) ----
            qsq = sb.tile([BS, C], f32)
            nc.scalar.activation(out=qsq[:, :], in_=dis_ps[:, 0:C], func=ACT.Square,
                                 bias=nxws[:, 0:1])
            m0 = sb.tile([BS, C], f32)
            nc.vector.scalar_tensor_tensor(
                out=m0[:, :], in0=qsq[:, :], scalar=ngam[:, 0:1], in1=dis_ps[:, C:2 * C],
                op0=ALU.mult, op1=ALU.add,
            )
            dis = sb.tile([BS, C], f32)
            nc.scalar.activation(out=dis[:, :], in_=m0[:, :], func=ACT.Sqrt,
                                 bias=s2[:, 0:1])

            # ---- loss_b = sum_c dis*mask2 via accumulate ----
            dj = sb.tile([BS, C], f32)
            loss = sb.tile([BS, 1], f32)
            nc.vector.scalar_tensor_tensor(
                out=dj[:, :], in0=dis[:, :], scalar=1.0, in1=mask2[:, :],
                op0=ALU.mult, op1=ALU.mult, accum_out=loss[:, :],
            )
            nc.sync.dma_start(loss_d[:, 0:1], loss[:, :])

    nc.finalize()
    return nc


def _get_nc():
    if "nc" not in _cache:
        _cache["nc"] = _build()
    return _cache["nc"]


def _host_precompute(feature_center):
    """Class-side constants in float64; folded into the matmul rhs.

    Layout [9, 209] (rows 0:8 = x contraction, row 8 = ones-row consts):
      cols 0:8    w  = M x - c0         rhs = [M      ; -c0     ]
      col  8      e0 = x.c0 - k0        rhs = [c0     ; -k0     ]
      cols 9:109  ur = x.r_c - kr_c     rhs = [r      ; -kr     ]
      cols 109:209 k2-2ur               rhs = [-2r    ; k2+2kr  ]
    """
    import ml_dtypes

    fc = np.asarray(feature_center, dtype=np.float64)
    g = fc.reshape(C, POOL, G).mean(axis=2)
    yn = g / (np.linalg.norm(g, axis=1, keepdims=True) + 1e-6)
    ybar = yn.mean(axis=0)
    z = yn - ybar
    A = (z.T @ z) / (2 * C - 1)
    M = np.linalg.inv(A)
    M = 0.5 * (M + M.T)
    r = yn @ M                                  # [100, 8]  M y_c
    c0 = M @ ybar
    k0 = float(ybar @ c0)
    k2 = np.einsum('cd,ce,de->c', z, z, M)
    kr = r @ ybar                               # ybar . r_c

    cp = np.zeros((POOL + 1, 209), dtype=np.float64)
    cp[0:POOL, 0:POOL] = M
    cp[POOL, 0:POOL] = -c0
    cp[0:POOL, POOL] = c0
    cp[POOL, POOL] = -k0
    cp[0:POOL, POOL + 1:POOL + 1 + C] = r.T
    cp[POOL, POOL + 1:POOL + 1 + C] = -kr
    cp[0:POOL, POOL + 1 + C:POOL + 1 + 2 * C] = -2.0 * r.T
    cp[POOL, POOL + 1 + C:POOL + 1 + 2 * C] = k2 + 2.0 * kr
    return cp.astype(ml_dtypes.bfloat16)


def kernel(hidden, feature_center, y):
    import ml_dtypes
    from concourse import bass_utils

    bf = ml_dtypes.bfloat16
    ha = np.empty((B, D + 1), dtype=bf)
    ha[:, 0:D] = np.asarray(hidden, dtype=np.float32).astype(bf)
    ha[:, D] = np.asarray(y).astype(bf)
    cp = _host_precompute(feature_center)

    nc = _get_nc()
    in_maps = []
    for c in range(NCORES):
        in_maps.append({
            "hidden_in": ha[c * BS:(c + 1) * BS],
            "const_in": cp,
        })
    res = bass_utils.run_bass_kernel_spmd(nc, in_maps, core_ids=list(range(NCORES)))
    loss = np.concatenate([r["loss_out"][:, 0] for r in res.results])
    return np.float32(loss.mean())
